# revision 73
# baseline (speedup 1.0000x reference)
"""CogVLM vision-expert attention on 8 Trainium2 NeuronCores.

Sharding: tensor-parallel over heads (4 heads per core). Each core gets
- replicated: hidden_states (transposed, bf16), RoPE tables, mask tiles
- sharded:    QKV weight columns + dense weight rows for its 4 heads
Each core computes q/k/v for its heads, head-local attention, and a
row-parallel partial of the dense output; the host sums the 8 partials.

Expert routing: tokens are host-sorted language-first, so the two experts
become an exact column split at the sorted boundary — every matmul computes
each token's single expert, no select/merge needed on device.

All matmul operands are bf16 (f32 PSUM accumulation). Attention runs in
transposed (S^T) orientation per block of 4 query tiles: scores land
[key, query] in PSUM, exp writes P^T straight to SBUF (no transposes, no
extra copies), row sums come from 1-row matmuls P_tile.T @ ones, and the
1/rowsum lands as a per-partition scale when ctx^T[i, d] is copied out.

Self-contained: hardcodes all shapes; only needs numpy + concourse.
"""

import numpy as np

B, S, H, NH = 1, 2048, 4096, 32
HD = H // NH          # 128
NCORES = 8
HPC = NH // NCORES    # 4 heads per core
NBLK = 3 * HPC        # 12 qkv col-blocks of 128 per core
NKT = H // 128        # 32 contraction tiles
NJT = S // 128        # 16 token tiles
ROPE_BASE = 10000.0

_CACHE = {}

# stage-B tuning knobs (engine: 'v'=DVE, 'g'=GPSIMD/Pool)
TUNE = {
    "sp_bufs": 4,      # PSUM score tiles in flight
    "cps_bufs": 2,
    "ctp_bufs": 1,
}


def _f32(x):
    return np.ascontiguousarray(x, dtype=np.float32)


def _build(ranges, mask_info, am_slots, n_am, dbg=False):
    """ranges: per 512-chunk tuple of (a, b, e) column ranges (exact expert
    split). mask_info[it][jt]: 0=compute, 1=compute+mask-add, 2=skip.
    am_slots: {(it, jt): slot} for mask-add tiles packed on host."""
    import concourse.bass as bass
    import concourse.mybir as mybir
    import concourse.tile as tile
    from concourse import bacc
    from contextlib import ExitStack
    import ml_dtypes

    dt = mybir.dt
    f32, bf16, f16 = dt.float32, dt.bfloat16, dt.float16
    AF = mybir.ActivationFunctionType
    AX = mybir.AxisListType.X

    nc = bacc.Bacc("TRN2", target_bir_lowering=False, debug=False)

    # DRAM inputs (host-packed layouts; see _host_prep)
    hs_d = nc.dram_tensor("hs", [128, NKT, S], bf16, kind="ExternalInput")
    wq_d = nc.dram_tensor("wqkv", [NBLK, 128, 2, NKT, 128], bf16,
                          kind="ExternalInput")
    wd_d = nc.dram_tensor("wdense", [32, 128, 2, HPC, 128], bf16,
                          kind="ExternalInput")
    cos_d = nc.dram_tensor("cosT", [HD, S], bf16, kind="ExternalInput")
    sin_d = nc.dram_tensor("sinT", [HD, S], bf16, kind="ExternalInput")
    am_d = nc.dram_tensor("amask", [128, max(n_am, 1), 128], dt.int8,
                          kind="ExternalInput")
    out_d = nc.dram_tensor("outT", [32, 128, S], bf16, kind="ExternalOutput")
    if dbg:
        qkv_dump = nc.dram_tensor("qkv_dump", [128, NBLK, S], bf16,
                                  kind="ExternalOutput")
        ctx_dump = nc.dram_tensor("ctx_dump", [HPC, 128, S], bf16,
                                  kind="ExternalOutput")

    eye_bf16 = nc.inline_tensor(np.eye(128, dtype=ml_dtypes.bfloat16),
                                "eye_bf16")
    # RT.T @ qT = rotate_half(q): row d<64 = -q[d+64], d>=64 = +q[d-64]
    RT_np = np.zeros((128, 128), dtype=ml_dtypes.bfloat16)
    for j in range(64):
        RT_np[j, j + 64] = 1.0
        RT_np[j + 64, j] = -1.0
    RT_t = nc.inline_tensor(RT_np, "RT")

    # per-it: maximal runs of non-skip tiles (for exp), and runs chopped to
    # <=8 jt (for PSUM score tiles)
    runs_of = []
    segs_of = []
    for it in range(NJT):
        runs = []
        segs = []
        j = 0
        while j < NJT:
            if mask_info[it][j] == 2:
                j += 1
                continue
            j0 = j
            while j < NJT and mask_info[it][j] != 2:
                j += 1
            runs.append((j0, j))
            for s0 in range(j0, j, 8):
                segs.append((s0, min(s0 + 8, j)))
        runs_of.append(runs)
        segs_of.append(segs)
    tiles_of = [[jt for jt in range(NJT) if mask_info[it][jt] != 2]
                for it in range(NJT)]
    # mask-add runs per it: consecutive mixed jts share consecutive slots
    am_runs = []
    for it in range(NJT):
        runs = []
        j = 0
        while j < NJT:
            if mask_info[it][j] != 1:
                j += 1
                continue
            j0 = j
            while j < NJT and mask_info[it][j] == 1:
                j += 1
            runs.append((j0, j, am_slots[(it, j0)]))
        am_runs.append(runs)

    with tile.TileContext(nc) as tc, ExitStack() as top:
        singles = top.enter_context(tc.tile_pool(name="singles", bufs=1))
        ident16 = singles.tile([128, 128], bf16)
        nc.sync.dma_start(out=ident16, in_=eye_bf16[:, :])
        RT_sb = singles.tile([128, 128], bf16)
        nc.sync.dma_start(out=RT_sb, in_=RT_t[:, :])
        nbias = singles.tile([128, 1], f32)
        nc.vector.memset(nbias, -24.0)

        # qkv (bf16) persists across stages A and B
        qkv_sb = singles.tile([128, NBLK, S], bf16, name="qkv_sb")

        # ---------------- Stage A: QKV projection (exact expert split) -----
        with ExitStack() as sa:
            pa = sa.enter_context(tc.tile_pool(name="qkv_sbuf", bufs=1))
            ppa = sa.enter_context(tc.tile_pool(name="qkv_psum", bufs=1,
                                                space="PSUM"))
            hs_sb = pa.tile([128, NKT, S], bf16, name="hs_sb")
            # 2D-sliced sub-DMAs (kt-groups x 512-token columns) in the
            # order the first chunk's kt-loop consumes them
            for tc_ in range(4):
                for kg in range(4):
                    nc.sync.dma_start(
                        out=hs_sb[:, kg * 8:(kg + 1) * 8,
                                  tc_ * 512:(tc_ + 1) * 512],
                        in_=hs_d[:, kg * 8:(kg + 1) * 8,
                                 tc_ * 512:(tc_ + 1) * 512])

            KH = NKT // 2  # weight tiles split in kt-halves to fit SBUF

            def load_w(nb):
                halves = []
                for kh in range(2):
                    w = pa.tile([128, 2, KH, 128], bf16, tag="w", bufs=3,
                                name=f"w_{nb}_{kh}")
                    nc.gpsimd.dma_start(
                        out=w, in_=wq_d[nb, :, :, kh * KH:(kh + 1) * KH, :])
                    halves.append(w)
                return halves

            next_w = load_w(0)
            for nb in range(NBLK):
                w = next_w
                if nb + 1 < NBLK:
                    next_w = load_w(nb + 1)
                for c in range(4):
                    ps = ppa.tile([128, 512], f32, tag="mmA", bufs=6,
                                  name=f"ps_{nb}_{c}")
                    for (a, b, e) in ranges[c]:
                        for kt in range(NKT):
                            nc.tensor.matmul(
                                ps[:, a:b],
                                lhsT=w[kt // KH][:, e, kt % KH, :],
                                rhs=hs_sb[:, kt, c * 512 + a:c * 512 + b],
                                start=(kt == 0), stop=(kt == NKT - 1),
                            )
                    nc.vector.tensor_copy(
                        out=qkv_sb[:, nb, c * 512:(c + 1) * 512], in_=ps)

        if dbg:
            nc.sync.dma_start(out=qkv_dump[:, :, :], in_=qkv_sb)

        # ctx tiles live across stages B and C (allocated after A frees hs)
        bc_pool = top.enter_context(tc.tile_pool(name="bc", bufs=1))
        ctxT = [bc_pool.tile([128, S], bf16, name=f"ctxT_{hl}")
                for hl in range(HPC)]

        # ---------------- Stage B: per-head attention ----------------------
        with ExitStack() as sb:
            pb = sb.enter_context(tc.tile_pool(name="att_sbuf", bufs=1))
            ppb = sb.enter_context(tc.tile_pool(name="att_psum", bufs=1,
                                                space="PSUM"))
            cos_sb = pb.tile([HD, S], bf16, name="cos_sb")
            nc.gpsimd.dma_start(out=cos_sb, in_=cos_d[:, :])
            sin_sb = pb.tile([HD, S], bf16, name="sin_sb")
            nc.gpsimd.dma_start(out=sin_sb, in_=sin_d[:, :])
            am_sb = None
            if n_am:
                am_sb = pb.tile([128, n_am, 128], dt.int8, name="am_sb")
                nc.gpsimd.dma_start(out=am_sb, in_=am_d[:, :, :])

            ENG = {"v": nc.vector, "g": nc.gpsimd}
            ei = [0]

            def rr(spec):  # round-robin over an engine spec string
                e = ENG[spec[ei[0] % len(spec)]]
                ei[0] += 1
                return e

            # RoPE: x' = x*cos + (RT.T @ x)*sin (scales folded into cos/sin
            # on host). All-bf16 DVE ops for 2x/4x DVE throughput. Head 0's
            # chunks are emitted upfront; heads 1-3's chunks interleave into
            # the previous head's it-loop so the DVE chain hides under PE.
            qkr = [(pb.tile([128, S], bf16, name=f"qr{hl}"),
                    pb.tile([128, S], bf16, name=f"kr{hl}"))
                   for hl in range(HPC)]

            def emit_rope_chunk(hl, xb, xr, ch, tag="cps"):
                cs = slice(ch * 512, ch * 512 + 512)
                if tag == "cps":
                    rot_t = ppb.tile([128, 4, 128], f32, tag="cps",
                                     bufs=TUNE["cps_bufs"],
                                     name=f"rot{hl}_{xb}_{ch}")
                    rot = rot_t[:, :, :]
                else:
                    rot = ppb.tile([128, 512], f32, tag="mmB",
                                   bufs=TUNE["sp_bufs"],
                                   name=f"rot{hl}_{xb}_{ch}")
                nc.tensor.matmul(rot, lhsT=RT_sb, rhs=qkv_sb[:, xb, cs],
                                 start=True, stop=True)
                m1 = pb.tile([128, 512], bf16, tag="ropetmp", bufs=3,
                             name=f"m1_{hl}_{xb}_{ch}")
                nc.vector.tensor_mul(out=m1, in0=qkv_sb[:, xb, cs],
                                     in1=cos_sb[:, cs])
                m2 = pb.tile([128, 512], bf16, tag="ropetmp2", bufs=3,
                             name=f"m2_{hl}_{xb}_{ch}")
                nc.vector.tensor_mul(out=m2, in0=rot, in1=sin_sb[:, cs])
                nc.vector.tensor_add(out=xr[:, cs], in0=m1, in1=m2)

            from collections import deque
            rope_q = deque()
            for hl in range(HPC):
                for xb, xr in ((3 * hl, qkr[hl][0]), (3 * hl + 1, qkr[hl][1])):
                    for ch in range(4):
                        rope_q.append((hl, xb, xr, ch))
            for _ in range(8):  # head 0 upfront: mmB tag is idle here
                emit_rope_chunk(*rope_q.popleft(), tag="mmB")

            ones_sb = pb.tile([128, 1], bf16, name="ones_col")
            nc.vector.memset(ones_sb, 1.0)
            zeros_sb = pb.tile([128, 4, 128], bf16, name="zeros_sb")
            nc.vector.memset(zeros_sb, 0.0)

            pending = [None]  # previous i-block awaiting PV/finish
            for hl in range(HPC):
                bq, bk, bv = 3 * hl, 3 * hl + 1, 3 * hl + 2
                qr, kr = qkr[hl]

                # v -> [t, d] layout via PE transpose; batches emitted
                # inside ig0's QK loop so they do not burst at head start
                v_sb = pb.tile([128, NJT, 128], bf16, tag="v_sb", bufs=2,
                               name=f"v{hl}")

                def emit_vt(jg, v_sb=v_sb, bv=bv):
                    vtp = ppb.tile([128, 4, 128], bf16, tag="ctp",
                                   bufs=TUNE["ctp_bufs"],
                                   name=f"vt{hl}_{jg}")
                    for j in range(4):
                        nc.tensor.transpose(
                            vtp[:, j, :],
                            qkv_sb[:, bv, (jg * 4 + j) * 128:
                                   (jg * 4 + j + 1) * 128],
                            ident16)
                    nc.vector.tensor_copy(
                        out=v_sb[:, jg * 4:(jg + 1) * 4, :], in_=vtp)

                # Attention in S^T orientation per block of 4 i-tiles:
                # scores land [j, i]; exp writes P^T straight to SBUF; row
                # sums via 1-row matmuls P_tile.T @ ones; 1/sum applied
                # per-partition when ctx^T[i, d] copies out; ctx then
                # re-transposed to [d, i]. The previous block's PV work is
                # emitted in il-sequential chunks inside the current block's
                # QK/exp phase (sequential PSUM accumulation groups).

                def emit_pv_il(st, il):
                    it = st["block"][il]
                    til = tiles_of[it]
                    for jt in til:
                        sl = st["slot"][jt]
                        nc.tensor.matmul(
                            st["rsum"][:, il:il + 1],
                            lhsT=st["pT_sb"][:, sl, il * 128:(il + 1) * 128],
                            rhs=ones_sb,
                            start=(jt == til[0]), stop=(jt == til[-1]))
                        nc.tensor.matmul(
                            st["cps"][:, il, :],
                            lhsT=st["pT_sb"][:, sl, il * 128:(il + 1) * 128],
                            rhs=st["v_sb"][:, jt, :],
                            start=(jt == til[0]), stop=(jt == til[-1]))

                def emit_finish(st):
                    hl_, ig_ = st["hl"], st["ig"]
                    rec_col = pb.tile([128, 4], f32, tag="rec", bufs=2,
                                      name=f"rec{hl_}_{ig_}")
                    nc.vector.reciprocal(out=rec_col, in_=st["rsum"])
                    ctxi = pb.tile([128, 4, 128], bf16, tag="ctxi", bufs=2,
                                   name=f"ctxi{hl_}_{ig_}")
                    for il in range(4):
                        nc.vector.tensor_scalar_mul(
                            out=ctxi[:, il, :], in0=st["cps"][:, il, :],
                            scalar1=rec_col[:, il:il + 1])
                    ctp = ppb.tile([128, 4, 128], bf16, tag="ctp",
                                   bufs=TUNE["ctp_bufs"],
                                   name=f"ctp{hl_}_{ig_}")
                    for il in range(4):
                        nc.tensor.transpose(ctp[:, il, :], ctxi[:, il, :],
                                            ident16)
                    nc.vector.tensor_copy(
                        out=ctxT[hl_][:, st["i0"]:st["i0"] + 512], in_=ctp)

                for ig in range(4):
                    block = [4 * ig + il for il in range(4)]
                    jts = sorted(set().union(
                        *[set(tiles_of[it]) for it in block]))
                    i0 = ig * 512
                    rsum = ppb.tile([128, 4], f32, tag="rsum", bufs=1,
                                    name=f"rsum{hl}_{ig}")
                    pT_sb = pb.tile([128, NJT, 512], bf16, tag="pT", bufs=2,
                                    name=f"pT{hl}_{ig}")
                    cps = ppb.tile([128, 4, 128], f32, tag="cps",
                                   bufs=TUNE["cps_bufs"],
                                   name=f"cps{hl}_{ig}")

                    def emit_qkT(k, jts=jts, block=block, i0=i0,
                                 pT_sb=pT_sb):
                        jt = jts[k]
                        # only the valid query-tile range needs computing;
                        # invalid slots inside it are never read downstream
                        valid = [il for il, it in enumerate(block)
                                 if mask_info[it][jt] != 2]
                        lo, hi = valid[0], valid[-1] + 1
                        spT = ppb.tile([128, 512], f32, tag="mmB",
                                       bufs=TUNE["sp_bufs"],
                                       name=f"spT{hl}_{ig}_{jt}")
                        nc.tensor.matmul(
                            spT[:, lo * 128:hi * 128],
                            lhsT=kr[:, jt * 128:(jt + 1) * 128],
                            rhs=qr[:, i0 + lo * 128:i0 + hi * 128],
                            start=True, stop=True)
                        nc.scalar.activation(
                            out=pT_sb[:, k, lo * 128:hi * 128],
                            in_=spT[:, lo * 128:hi * 128], func=AF.Exp,
                            bias=nbias, scale=1.0)
                        # zero masked entries after exp (mask tiles are
                        # nonzero exactly where masked); off the spT chain
                        il = 0
                        while il < 4:
                            if mask_info[block[il]][jt] != 1:
                                il += 1
                                continue
                            a = il
                            while il < 4 and mask_info[block[il]][jt] == 1:
                                il += 1
                            slot = am_slots[(block[a], jt)]
                            nc.vector.copy_predicated(
                                out=pT_sb[:, k, a * 128:il * 128],
                                mask=am_sb[:, slot:slot + (il - a), :],
                                data=zeros_sb[:, :il - a, :])

                    quarter = max(1, (len(jts) + 3) // 4)
                    pvil = 0
                    for k in range(len(jts)):
                        emit_qkT(k)
                        if ig == 0 and k < 4:
                            emit_vt(k)
                        if k % 4 == 1 and rope_q and rope_q[0][0] == hl + 1:
                            emit_rope_chunk(*rope_q.popleft())
                        if (pending[0] is not None and pvil < 4
                                and (k + 1) % quarter == 0):
                            emit_pv_il(pending[0], pvil)
                            pvil += 1
                    if ig == 0:
                        for jg in range(len(jts), 4):
                            emit_vt(jg)
                    if pending[0] is not None:
                        while pvil < 4:
                            emit_pv_il(pending[0], pvil)
                            pvil += 1
                        emit_finish(pending[0])
                    pending[0] = dict(hl=hl, ig=ig, block=block, jts=jts,
                                      i0=i0, rsum=rsum, pT_sb=pT_sb,
                                      cps=cps, v_sb=v_sb,
                                      slot={jt: kk for kk, jt
                                            in enumerate(jts)})

            # flush the last block
            st = pending[0]
            for il in range(4):
                emit_pv_il(st, il)
            emit_finish(st)
            pending[0] = None

        if dbg:
            for hl in range(HPC):
                nc.sync.dma_start(out=ctx_dump[hl], in_=ctxT[hl])

        # ---------------- Stage C: row-parallel dense (exact split) --------
        with ExitStack() as sc:
            # C's SBUF tiles live in bc_pool (allocated before stage B claims
            # space): no WAR against B's tail readers, prefetch starts early
            pc = bc_pool
            ppc = sc.enter_context(tc.tile_pool(name="dense_psum", bufs=1,
                                                space="PSUM"))

            def load_wd(nb):
                wd = pc.tile([128, 2, HPC, 128], bf16, tag="wd", bufs=6,
                             name=f"wd_{nb}")
                nc.gpsimd.dma_start(out=wd, in_=wd_d[nb])
                return wd

            next_wd = load_wd(0)
            for nb in range(32):
                wd = next_wd
                if nb + 1 < 32:
                    next_wd = load_wd(nb + 1)
                ob = pc.tile([128, S], bf16, tag="ob", bufs=3,
                             name=f"ob_{nb}")
                for c in range(4):
                    ops = ppc.tile([128, 512], f32, tag="mmC", bufs=6,
                                   name=f"o_{nb}_{c}")
                    for (a, b, e) in ranges[c]:
                        for dt_ in range(HPC):
                            nc.tensor.matmul(
                                ops[:, a:b],
                                lhsT=wd[:, e, dt_, :],
                                rhs=ctxT[dt_][:, c * 512 + a:c * 512 + b],
                                start=(dt_ == 0), stop=(dt_ == HPC - 1))
                    nc.scalar.activation(
                        out=ob[:, c * 512:(c + 1) * 512], in_=ops,
                        func=AF.Copy, bias=0.0, scale=1.0)
                nc.gpsimd.dma_start(out=out_d[nb], in_=ob)

    nc.finalize()
    return nc


def _host_prep(inputs):
    import ml_dtypes

    hs = _f32(np.asarray(inputs["hidden_states"])).reshape(S, H)
    tt = np.asarray(inputs["token_type_ids"]).reshape(S)
    pos = np.asarray(inputs["position_ids"]).reshape(S).astype(np.int64)
    am = _f32(np.asarray(inputs["attention_mask"])).reshape(
        np.asarray(inputs["attention_mask"]).shape[-2], -1
    )[:S, :S]
    wv_qkv = _f32(inputs["wv_qkv"])
    wl_qkv = _f32(inputs["wl_qkv"])
    wv_dense = _f32(inputs["wv_dense"])
    wl_dense = _f32(inputs["wl_dense"])

    # routing mask: vision iff tt[i]==1 and tt[i+1]==1; last position language
    core = (tt[:-1] == 1) & (tt[1:] == 1)
    vmb = np.concatenate([core, [False]])

    # sort tokens: language first, stable; attention uses the permuted mask
    perm = np.argsort(vmb, kind="stable")
    vmb_p = vmb[perm]
    nl = int((~vmb_p).sum())  # tokens [0, nl) language (expert 1), rest vision
    hs_p = hs[perm]
    pos_p = pos[perm]
    am_p = np.ascontiguousarray(am[np.ix_(perm, perm)])

    # exact expert column ranges per 512-token chunk (e: 0=vision, 1=language)
    ranges = []
    for c in range(4):
        lo, hi = 512 * c, 512 * (c + 1)
        if hi <= nl:
            ranges.append(((0, 512, 1),))
        elif lo >= nl:
            ranges.append(((0, 512, 0),))
        else:
            ranges.append(((0, nl - lo, 1), (nl - lo, 512, 0)))
    ranges = tuple(ranges)

    inv_freq = 1.0 / (ROPE_BASE ** (np.arange(0, HD, 2, dtype=np.float32) / HD))
    t = np.arange(S, dtype=np.float32)
    emb = np.concatenate([np.outer(t, inv_freq)] * 2, axis=-1)  # [S, HD]
    ss = np.float32(np.sqrt(1.0 / np.sqrt(HD)))
    cosT = np.ascontiguousarray(
        (np.cos(emb).astype(np.float32) * ss)[pos_p].T).astype(
        ml_dtypes.bfloat16)  # [HD, S]
    sinT = np.ascontiguousarray(
        (np.sin(emb).astype(np.float32) * ss)[pos_p].T).astype(
        ml_dtypes.bfloat16)

    # per-(i-tile, j-tile) mask: 0=all-zero, 1=mixed (add), 2=all-masked (skip)
    mask_info = []
    for it in range(NJT):
        row = []
        for jt in range(NJT):
            blk = am_p[it * 128:(it + 1) * 128, jt * 128:(jt + 1) * 128]
            if blk.max() < -1e8:
                row.append(2)
            elif blk.min() == 0.0 and blk.max() == 0.0:
                row.append(0)
            else:
                row.append(1)
        if all(s == 2 for s in row):
            row[it] = 1  # fully-masked row: keep diagonal for a valid softmax
        mask_info.append(tuple(row))
    mask_info = tuple(mask_info)

    # pack mask-add tiles in (block, jt, il) order: a block's mixed tiles
    # for one key-tile are consecutive slots -> single wide DVE add
    am_slots = {}
    strips = []
    slot = 0
    for ig in range(4):
        for jt in range(NJT):
            for il in range(4):
                it = 4 * ig + il
                if mask_info[it][jt] == 1:
                    am_slots[(it, jt)] = slot
                    blk = am_p[it * 128:(it + 1) * 128,
                               jt * 128:(jt + 1) * 128]
                    # TRANSPOSED int8 indicator (1 = masked), [j, i] layout
                    strips.append(np.ascontiguousarray(
                        (blk.T < -1e8)).astype(np.int8))
                    slot += 1
    n_am = slot
    if n_am:
        am_np = np.ascontiguousarray(
            np.stack(strips, axis=1))  # [128, n_am, 128]
    else:
        am_np = np.zeros((128, 1, 128), dtype=np.int8)
    am_slots = tuple(sorted(am_slots.items()))

    # hidden states: [p, kt, t] bf16
    hs_np = np.ascontiguousarray(
        hs_p.T.reshape(NKT, 128, S).transpose(1, 0, 2)
    ).astype(ml_dtypes.bfloat16)

    in_maps = []
    for cid in range(NCORES):
        # wqkv tile layout [nb, p, e, kt, n]; nb = 3*hl + part
        wq_np = np.empty((NBLK, 128, 2, NKT, 128), dtype=ml_dtypes.bfloat16)
        for hl in range(HPC):
            h = HPC * cid + hl
            for part in range(3):
                nb = 3 * hl + part
                col0 = part * H + h * HD
                for e, w in ((0, wv_qkv), (1, wl_qkv)):
                    blk = w[:, col0:col0 + HD]  # [4096, 128]
                    wq_np[nb, :, e] = blk.reshape(NKT, 128, 128).transpose(
                        1, 0, 2).astype(ml_dtypes.bfloat16)
        # wdense tile layout [nb, p, e, dt, n]
        r0 = 512 * cid
        wd_np = np.empty((32, 128, 2, HPC, 128), dtype=ml_dtypes.bfloat16)
        for e, w in ((0, wv_dense), (1, wl_dense)):
            blk = w[r0:r0 + 512]  # [512, 4096]
            wd_np[:, :, e] = blk.reshape(HPC, 128, 32, 128).transpose(
                2, 1, 0, 3).astype(ml_dtypes.bfloat16)
        in_maps.append({
            "hs": hs_np,
            "wqkv": np.ascontiguousarray(wq_np),
            "wdense": np.ascontiguousarray(wd_np),
            "cosT": cosT,
            "sinT": sinT,
            "amask": am_np,
        })
    key = (ranges, mask_info, am_slots, n_am)
    return key, perm, in_maps


PROFILE = False
LAST_EXEC_NS = None
LAST_RESULTS = None


def kernel(**inputs):
    global LAST_EXEC_NS, LAST_RESULTS
    from concourse.bass_utils import run_bass_kernel_spmd

    key, perm, in_maps = _host_prep(inputs)
    bkey = (key[0], key[1], key[3])
    if bkey not in _CACHE:
        am_slots = dict(key[2])
        _CACHE[bkey] = _build(key[0], key[1], am_slots, key[3])
    nc = _CACHE[bkey]
    kw = {}
    if PROFILE:
        try:
            import antenv.axon_hooks  # noqa: F401
            kw = {"trace": True}
        except ImportError:
            pass
    res = run_bass_kernel_spmd(nc, in_maps, core_ids=list(range(NCORES)), **kw)
    LAST_EXEC_NS = res.exec_time_ns
    LAST_RESULTS = res
    acc = np.zeros((32, 128, S), dtype=np.float32)
    for r in res.results:
        acc += np.asarray(r["outT"], dtype=np.float32)
    full = acc.reshape(H, S)  # [h, t]
    out = np.empty((S, H), dtype=np.float32)
    out[perm] = full.T
    return np.ascontiguousarray(out).reshape(B, S, H)


# revision 74
# speedup vs baseline: 1.0002x; 1.0002x over previous
"""CogVLM vision-expert attention on 8 Trainium2 NeuronCores.

Sharding: tensor-parallel over heads (4 heads per core). Each core gets
- replicated: hidden_states (transposed, bf16), RoPE tables, mask tiles
- sharded:    QKV weight columns + dense weight rows for its 4 heads
Each core computes q/k/v for its heads, head-local attention, and a
row-parallel partial of the dense output; the host sums the 8 partials.

Expert routing: tokens are host-sorted language-first, so the two experts
become an exact column split at the sorted boundary — every matmul computes
each token's single expert, no select/merge needed on device.

All matmul operands are bf16 (f32 PSUM accumulation). Attention runs in
transposed (S^T) orientation per block of 4 query tiles: scores land
[key, query] in PSUM, exp writes P^T straight to SBUF (no transposes, no
extra copies), row sums come from 1-row matmuls P_tile.T @ ones, and the
1/rowsum lands as a per-partition scale when ctx^T[i, d] is copied out.

Self-contained: hardcodes all shapes; only needs numpy + concourse.
"""

import numpy as np

B, S, H, NH = 1, 2048, 4096, 32
HD = H // NH          # 128
NCORES = 8
HPC = NH // NCORES    # 4 heads per core
NBLK = 3 * HPC        # 12 qkv col-blocks of 128 per core
NKT = H // 128        # 32 contraction tiles
NJT = S // 128        # 16 token tiles
ROPE_BASE = 10000.0

_CACHE = {}

# stage-B tuning knobs (engine: 'v'=DVE, 'g'=GPSIMD/Pool)
TUNE = {
    "sp_bufs": 4,      # PSUM score tiles in flight
    "cps_bufs": 2,
    "ctp_bufs": 1,
}


def _f32(x):
    return np.ascontiguousarray(x, dtype=np.float32)


def _build(ranges, mask_info, am_slots, n_am, dbg=False):
    """ranges: per 512-chunk tuple of (a, b, e) column ranges (exact expert
    split). mask_info[it][jt]: 0=compute, 1=compute+mask-add, 2=skip.
    am_slots: {(it, jt): slot} for mask-add tiles packed on host."""
    import concourse.bass as bass
    import concourse.mybir as mybir
    import concourse.tile as tile
    from concourse import bacc
    from contextlib import ExitStack
    import ml_dtypes

    dt = mybir.dt
    f32, bf16, f16 = dt.float32, dt.bfloat16, dt.float16
    AF = mybir.ActivationFunctionType
    AX = mybir.AxisListType.X

    nc = bacc.Bacc("TRN2", target_bir_lowering=False, debug=False)

    # DRAM inputs (host-packed layouts; see _host_prep)
    hs_d = nc.dram_tensor("hs", [128, NKT, S], bf16, kind="ExternalInput")
    wq_d = nc.dram_tensor("wqkv", [NBLK, 128, 2, NKT, 128], bf16,
                          kind="ExternalInput")
    wd_d = nc.dram_tensor("wdense", [32, 128, 2, HPC, 128], bf16,
                          kind="ExternalInput")
    cos_d = nc.dram_tensor("cosT", [HD, S], bf16, kind="ExternalInput")
    sin_d = nc.dram_tensor("sinT", [HD, S], bf16, kind="ExternalInput")
    am_d = nc.dram_tensor("amask", [128, max(n_am, 1), 128], dt.int8,
                          kind="ExternalInput")
    out_d = nc.dram_tensor("outT", [32, 128, S], bf16, kind="ExternalOutput")
    if dbg:
        qkv_dump = nc.dram_tensor("qkv_dump", [128, NBLK, S], bf16,
                                  kind="ExternalOutput")
        ctx_dump = nc.dram_tensor("ctx_dump", [HPC, 128, S], bf16,
                                  kind="ExternalOutput")

    eye_bf16 = nc.inline_tensor(np.eye(128, dtype=ml_dtypes.bfloat16),
                                "eye_bf16")
    # RT.T @ qT = rotate_half(q): row d<64 = -q[d+64], d>=64 = +q[d-64]
    RT_np = np.zeros((128, 128), dtype=ml_dtypes.bfloat16)
    for j in range(64):
        RT_np[j, j + 64] = 1.0
        RT_np[j + 64, j] = -1.0
    RT_t = nc.inline_tensor(RT_np, "RT")

    # per-it: maximal runs of non-skip tiles (for exp), and runs chopped to
    # <=8 jt (for PSUM score tiles)
    runs_of = []
    segs_of = []
    for it in range(NJT):
        runs = []
        segs = []
        j = 0
        while j < NJT:
            if mask_info[it][j] == 2:
                j += 1
                continue
            j0 = j
            while j < NJT and mask_info[it][j] != 2:
                j += 1
            runs.append((j0, j))
            for s0 in range(j0, j, 8):
                segs.append((s0, min(s0 + 8, j)))
        runs_of.append(runs)
        segs_of.append(segs)
    tiles_of = [[jt for jt in range(NJT) if mask_info[it][jt] != 2]
                for it in range(NJT)]
    # mask-add runs per it: consecutive mixed jts share consecutive slots
    am_runs = []
    for it in range(NJT):
        runs = []
        j = 0
        while j < NJT:
            if mask_info[it][j] != 1:
                j += 1
                continue
            j0 = j
            while j < NJT and mask_info[it][j] == 1:
                j += 1
            runs.append((j0, j, am_slots[(it, j0)]))
        am_runs.append(runs)

    with tile.TileContext(nc) as tc, ExitStack() as top:
        singles = top.enter_context(tc.tile_pool(name="singles", bufs=1))
        ident16 = singles.tile([128, 128], bf16)
        nc.sync.dma_start(out=ident16, in_=eye_bf16[:, :])
        RT_sb = singles.tile([128, 128], bf16)
        nc.sync.dma_start(out=RT_sb, in_=RT_t[:, :])
        nbias = singles.tile([128, 1], f32)
        nc.vector.memset(nbias, -24.0)

        # qkv (bf16) persists across stages A and B
        qkv_sb = singles.tile([128, NBLK, S], bf16, name="qkv_sb")

        # ---------------- Stage A: QKV projection (exact expert split) -----
        with ExitStack() as sa:
            pa = sa.enter_context(tc.tile_pool(name="qkv_sbuf", bufs=1))
            ppa = sa.enter_context(tc.tile_pool(name="qkv_psum", bufs=1,
                                                space="PSUM"))
            hs_sb = pa.tile([128, NKT, S], bf16, name="hs_sb")
            # 2D-sliced sub-DMAs (kt-groups x 512-token columns) in the
            # order the first chunk's kt-loop consumes them
            for tc_ in range(4):
                for kg in range(4):
                    nc.sync.dma_start(
                        out=hs_sb[:, kg * 8:(kg + 1) * 8,
                                  tc_ * 512:(tc_ + 1) * 512],
                        in_=hs_d[:, kg * 8:(kg + 1) * 8,
                                 tc_ * 512:(tc_ + 1) * 512])

            KH = NKT // 2  # weight tiles split in kt-halves to fit SBUF

            def load_w(nb):
                halves = []
                for kh in range(2):
                    w = pa.tile([128, 2, KH, 128], bf16, tag="w", bufs=3,
                                name=f"w_{nb}_{kh}")
                    nc.gpsimd.dma_start(
                        out=w, in_=wq_d[nb, :, :, kh * KH:(kh + 1) * KH, :])
                    halves.append(w)
                return halves

            next_w = load_w(0)
            for nb in range(NBLK):
                w = next_w
                if nb + 1 < NBLK:
                    next_w = load_w(nb + 1)
                for c in range(4):
                    ps = ppa.tile([128, 512], f32, tag="mmA", bufs=6,
                                  name=f"ps_{nb}_{c}")
                    for (a, b, e) in ranges[c]:
                        for kt in range(NKT):
                            nc.tensor.matmul(
                                ps[:, a:b],
                                lhsT=w[kt // KH][:, e, kt % KH, :],
                                rhs=hs_sb[:, kt, c * 512 + a:c * 512 + b],
                                start=(kt == 0), stop=(kt == NKT - 1),
                            )
                    nc.vector.tensor_copy(
                        out=qkv_sb[:, nb, c * 512:(c + 1) * 512], in_=ps)

        if dbg:
            nc.sync.dma_start(out=qkv_dump[:, :, :], in_=qkv_sb)

        # ctx tiles live across stages B and C (allocated after A frees hs)
        bc_pool = top.enter_context(tc.tile_pool(name="bc", bufs=1))
        ctxT = [bc_pool.tile([128, S], bf16, name=f"ctxT_{hl}")
                for hl in range(HPC)]

        # ---------------- Stage B: per-head attention ----------------------
        with ExitStack() as sb:
            pb = sb.enter_context(tc.tile_pool(name="att_sbuf", bufs=1))
            ppb = sb.enter_context(tc.tile_pool(name="att_psum", bufs=1,
                                                space="PSUM"))
            cos_sb = pb.tile([HD, S], bf16, name="cos_sb")
            nc.gpsimd.dma_start(out=cos_sb, in_=cos_d[:, :])
            sin_sb = pb.tile([HD, S], bf16, name="sin_sb")
            nc.gpsimd.dma_start(out=sin_sb, in_=sin_d[:, :])
            am_sb = None
            if n_am:
                am_sb = pb.tile([128, n_am, 128], dt.int8, name="am_sb")
                nc.gpsimd.dma_start(out=am_sb, in_=am_d[:, :, :])

            ENG = {"v": nc.vector, "g": nc.gpsimd}
            ei = [0]

            def rr(spec):  # round-robin over an engine spec string
                e = ENG[spec[ei[0] % len(spec)]]
                ei[0] += 1
                return e

            # RoPE: x' = x*cos + (RT.T @ x)*sin (scales folded into cos/sin
            # on host). All-bf16 DVE ops for 2x/4x DVE throughput. Head 0's
            # chunks are emitted upfront; heads 1-3's chunks interleave into
            # the previous head's it-loop so the DVE chain hides under PE.
            qkr = [(pb.tile([128, S], bf16, name=f"qr{hl}"),
                    pb.tile([128, S], bf16, name=f"kr{hl}"))
                   for hl in range(HPC)]

            def emit_rope_chunk(hl, xb, xr, ch, tag="cps"):
                cs = slice(ch * 512, ch * 512 + 512)
                if tag == "cps":
                    rot_t = ppb.tile([128, 4, 128], f32, tag="cps",
                                     bufs=TUNE["cps_bufs"],
                                     name=f"rot{hl}_{xb}_{ch}")
                    rot = rot_t[:, :, :]
                else:
                    rot = ppb.tile([128, 512], f32, tag="mmB",
                                   bufs=TUNE["sp_bufs"],
                                   name=f"rot{hl}_{xb}_{ch}")
                nc.tensor.matmul(rot, lhsT=RT_sb, rhs=qkv_sb[:, xb, cs],
                                 start=True, stop=True)
                m1 = pb.tile([128, 512], bf16, tag="ropetmp", bufs=3,
                             name=f"m1_{hl}_{xb}_{ch}")
                nc.vector.tensor_mul(out=m1, in0=qkv_sb[:, xb, cs],
                                     in1=cos_sb[:, cs])
                m2 = pb.tile([128, 512], bf16, tag="ropetmp2", bufs=3,
                             name=f"m2_{hl}_{xb}_{ch}")
                nc.vector.tensor_mul(out=m2, in0=rot, in1=sin_sb[:, cs])
                nc.vector.tensor_add(out=xr[:, cs], in0=m1, in1=m2)

            from collections import deque
            rope_q = deque()
            for hl in range(HPC):
                for xb, xr in ((3 * hl, qkr[hl][0]), (3 * hl + 1, qkr[hl][1])):
                    for ch in range(4):
                        rope_q.append((hl, xb, xr, ch))
            for _ in range(8):  # head 0 upfront: mmB tag is idle here
                emit_rope_chunk(*rope_q.popleft(), tag="mmB")

            ones_sb = pb.tile([128, 1], bf16, name="ones_col")
            nc.vector.memset(ones_sb, 1.0)
            zeros_sb = pb.tile([128, 4, 128], bf16, name="zeros_sb")
            nc.vector.memset(zeros_sb, 0.0)

            pending = [None]  # previous i-block awaiting PV/finish
            for hl in range(HPC):
                bq, bk, bv = 3 * hl, 3 * hl + 1, 3 * hl + 2
                qr, kr = qkr[hl]

                # v -> [t, d] layout via PE transpose; batches emitted
                # inside ig0's QK loop so they do not burst at head start
                v_sb = pb.tile([128, NJT, 128], bf16, tag="v_sb", bufs=2,
                               name=f"v{hl}")

                def emit_vt(jg, v_sb=v_sb, bv=bv):
                    vtp = ppb.tile([128, 4, 128], bf16, tag="ctp",
                                   bufs=TUNE["ctp_bufs"],
                                   name=f"vt{hl}_{jg}")
                    for j in range(4):
                        nc.tensor.transpose(
                            vtp[:, j, :],
                            qkv_sb[:, bv, (jg * 4 + j) * 128:
                                   (jg * 4 + j + 1) * 128],
                            ident16)
                    nc.vector.tensor_copy(
                        out=v_sb[:, jg * 4:(jg + 1) * 4, :], in_=vtp)

                # Attention in S^T orientation per block of 4 i-tiles:
                # scores land [j, i]; exp writes P^T straight to SBUF; row
                # sums via 1-row matmuls P_tile.T @ ones; 1/sum applied
                # per-partition when ctx^T[i, d] copies out; ctx then
                # re-transposed to [d, i]. The previous block's PV work is
                # emitted in il-sequential chunks inside the current block's
                # QK/exp phase (sequential PSUM accumulation groups).

                def emit_pv_op(st, il, jt):
                    til = tiles_of[st["block"][il]]
                    sl = st["slot"][jt]
                    nc.tensor.matmul(
                        st["rsum"][:, il:il + 1],
                        lhsT=st["pT_sb"][:, sl, il * 128:(il + 1) * 128],
                        rhs=ones_sb,
                        start=(jt == til[0]), stop=(jt == til[-1]))
                    nc.tensor.matmul(
                        st["cps"][:, il, :],
                        lhsT=st["pT_sb"][:, sl, il * 128:(il + 1) * 128],
                        rhs=st["v_sb"][:, jt, :],
                        start=(jt == til[0]), stop=(jt == til[-1]))

                def pv_ops_of(st):
                    return [(il, jt) for il in range(4)
                            for jt in tiles_of[st["block"][il]]]

                def emit_finish(st):
                    hl_, ig_ = st["hl"], st["ig"]
                    rec_col = pb.tile([128, 4], f32, tag="rec", bufs=2,
                                      name=f"rec{hl_}_{ig_}")
                    nc.vector.reciprocal(out=rec_col, in_=st["rsum"])
                    ctxi = pb.tile([128, 4, 128], bf16, tag="ctxi", bufs=2,
                                   name=f"ctxi{hl_}_{ig_}")
                    for il in range(4):
                        nc.vector.tensor_scalar_mul(
                            out=ctxi[:, il, :], in0=st["cps"][:, il, :],
                            scalar1=rec_col[:, il:il + 1])
                    ctp = ppb.tile([128, 4, 128], bf16, tag="ctp",
                                   bufs=TUNE["ctp_bufs"],
                                   name=f"ctp{hl_}_{ig_}")
                    for il in range(4):
                        nc.tensor.transpose(ctp[:, il, :], ctxi[:, il, :],
                                            ident16)
                    nc.vector.tensor_copy(
                        out=ctxT[hl_][:, st["i0"]:st["i0"] + 512], in_=ctp)

                for ig in range(4):
                    block = [4 * ig + il for il in range(4)]
                    jts = sorted(set().union(
                        *[set(tiles_of[it]) for it in block]))
                    i0 = ig * 512
                    rsum = ppb.tile([128, 4], f32, tag="rsum", bufs=1,
                                    name=f"rsum{hl}_{ig}")
                    pT_sb = pb.tile([128, NJT, 512], bf16, tag="pT", bufs=2,
                                    name=f"pT{hl}_{ig}")
                    cps = ppb.tile([128, 4, 128], f32, tag="cps",
                                   bufs=TUNE["cps_bufs"],
                                   name=f"cps{hl}_{ig}")

                    def emit_qkT(k, jts=jts, block=block, i0=i0,
                                 pT_sb=pT_sb):
                        jt = jts[k]
                        # only the valid query-tile range needs computing;
                        # invalid slots inside it are never read downstream
                        valid = [il for il, it in enumerate(block)
                                 if mask_info[it][jt] != 2]
                        lo, hi = valid[0], valid[-1] + 1
                        spT = ppb.tile([128, 512], f32, tag="mmB",
                                       bufs=TUNE["sp_bufs"],
                                       name=f"spT{hl}_{ig}_{jt}")
                        nc.tensor.matmul(
                            spT[:, lo * 128:hi * 128],
                            lhsT=kr[:, jt * 128:(jt + 1) * 128],
                            rhs=qr[:, i0 + lo * 128:i0 + hi * 128],
                            start=True, stop=True)
                        nc.scalar.activation(
                            out=pT_sb[:, k, lo * 128:hi * 128],
                            in_=spT[:, lo * 128:hi * 128], func=AF.Exp,
                            bias=nbias, scale=1.0)
                        # zero masked entries after exp (mask tiles are
                        # nonzero exactly where masked); off the spT chain
                        il = 0
                        while il < 4:
                            if mask_info[block[il]][jt] != 1:
                                il += 1
                                continue
                            a = il
                            while il < 4 and mask_info[block[il]][jt] == 1:
                                il += 1
                            slot = am_slots[(block[a], jt)]
                            nc.vector.copy_predicated(
                                out=pT_sb[:, k, a * 128:il * 128],
                                mask=am_sb[:, slot:slot + (il - a), :],
                                data=zeros_sb[:, :il - a, :])

                    pvq = pv_ops_of(pending[0]) if pending[0] else []
                    per_k = max(1, -(-len(pvq) // max(1, len(jts))))
                    pi = 0
                    for k in range(len(jts)):
                        emit_qkT(k)
                        if ig == 0 and k < 4:
                            emit_vt(k)
                        if k % 4 == 1 and rope_q and rope_q[0][0] == hl + 1:
                            emit_rope_chunk(*rope_q.popleft())
                        for _ in range(per_k):
                            if pi < len(pvq):
                                emit_pv_op(pending[0], *pvq[pi])
                                pi += 1
                    if ig == 0:
                        for jg in range(len(jts), 4):
                            emit_vt(jg)
                    if pending[0] is not None:
                        while pi < len(pvq):
                            emit_pv_op(pending[0], *pvq[pi])
                            pi += 1
                        emit_finish(pending[0])
                    pending[0] = dict(hl=hl, ig=ig, block=block, jts=jts,
                                      i0=i0, rsum=rsum, pT_sb=pT_sb,
                                      cps=cps, v_sb=v_sb,
                                      slot={jt: kk for kk, jt
                                            in enumerate(jts)})

            # flush the last block
            st = pending[0]
            for il, jt in pv_ops_of(st):
                emit_pv_op(st, il, jt)
            emit_finish(st)
            pending[0] = None

        if dbg:
            for hl in range(HPC):
                nc.sync.dma_start(out=ctx_dump[hl], in_=ctxT[hl])

        # ---------------- Stage C: row-parallel dense (exact split) --------
        with ExitStack() as sc:
            # C's SBUF tiles live in bc_pool (allocated before stage B claims
            # space): no WAR against B's tail readers, prefetch starts early
            pc = bc_pool
            ppc = sc.enter_context(tc.tile_pool(name="dense_psum", bufs=1,
                                                space="PSUM"))

            def load_wd(nb):
                wd = pc.tile([128, 2, HPC, 128], bf16, tag="wd", bufs=6,
                             name=f"wd_{nb}")
                nc.gpsimd.dma_start(out=wd, in_=wd_d[nb])
                return wd

            next_wd = load_wd(0)
            for nb in range(32):
                wd = next_wd
                if nb + 1 < 32:
                    next_wd = load_wd(nb + 1)
                ob = pc.tile([128, S], bf16, tag="ob", bufs=3,
                             name=f"ob_{nb}")
                for c in range(4):
                    ops = ppc.tile([128, 512], f32, tag="mmC", bufs=6,
                                   name=f"o_{nb}_{c}")
                    for (a, b, e) in ranges[c]:
                        for dt_ in range(HPC):
                            nc.tensor.matmul(
                                ops[:, a:b],
                                lhsT=wd[:, e, dt_, :],
                                rhs=ctxT[dt_][:, c * 512 + a:c * 512 + b],
                                start=(dt_ == 0), stop=(dt_ == HPC - 1))
                    nc.scalar.activation(
                        out=ob[:, c * 512:(c + 1) * 512], in_=ops,
                        func=AF.Copy, bias=0.0, scale=1.0)
                nc.gpsimd.dma_start(out=out_d[nb], in_=ob)

    nc.finalize()
    return nc


def _host_prep(inputs):
    import ml_dtypes

    hs = _f32(np.asarray(inputs["hidden_states"])).reshape(S, H)
    tt = np.asarray(inputs["token_type_ids"]).reshape(S)
    pos = np.asarray(inputs["position_ids"]).reshape(S).astype(np.int64)
    am = _f32(np.asarray(inputs["attention_mask"])).reshape(
        np.asarray(inputs["attention_mask"]).shape[-2], -1
    )[:S, :S]
    wv_qkv = _f32(inputs["wv_qkv"])
    wl_qkv = _f32(inputs["wl_qkv"])
    wv_dense = _f32(inputs["wv_dense"])
    wl_dense = _f32(inputs["wl_dense"])

    # routing mask: vision iff tt[i]==1 and tt[i+1]==1; last position language
    core = (tt[:-1] == 1) & (tt[1:] == 1)
    vmb = np.concatenate([core, [False]])

    # sort tokens: language first, stable; attention uses the permuted mask
    perm = np.argsort(vmb, kind="stable")
    vmb_p = vmb[perm]
    nl = int((~vmb_p).sum())  # tokens [0, nl) language (expert 1), rest vision
    hs_p = hs[perm]
    pos_p = pos[perm]
    am_p = np.ascontiguousarray(am[np.ix_(perm, perm)])

    # exact expert column ranges per 512-token chunk (e: 0=vision, 1=language)
    ranges = []
    for c in range(4):
        lo, hi = 512 * c, 512 * (c + 1)
        if hi <= nl:
            ranges.append(((0, 512, 1),))
        elif lo >= nl:
            ranges.append(((0, 512, 0),))
        else:
            ranges.append(((0, nl - lo, 1), (nl - lo, 512, 0)))
    ranges = tuple(ranges)

    inv_freq = 1.0 / (ROPE_BASE ** (np.arange(0, HD, 2, dtype=np.float32) / HD))
    t = np.arange(S, dtype=np.float32)
    emb = np.concatenate([np.outer(t, inv_freq)] * 2, axis=-1)  # [S, HD]
    ss = np.float32(np.sqrt(1.0 / np.sqrt(HD)))
    cosT = np.ascontiguousarray(
        (np.cos(emb).astype(np.float32) * ss)[pos_p].T).astype(
        ml_dtypes.bfloat16)  # [HD, S]
    sinT = np.ascontiguousarray(
        (np.sin(emb).astype(np.float32) * ss)[pos_p].T).astype(
        ml_dtypes.bfloat16)

    # per-(i-tile, j-tile) mask: 0=all-zero, 1=mixed (add), 2=all-masked (skip)
    mask_info = []
    for it in range(NJT):
        row = []
        for jt in range(NJT):
            blk = am_p[it * 128:(it + 1) * 128, jt * 128:(jt + 1) * 128]
            if blk.max() < -1e8:
                row.append(2)
            elif blk.min() == 0.0 and blk.max() == 0.0:
                row.append(0)
            else:
                row.append(1)
        if all(s == 2 for s in row):
            row[it] = 1  # fully-masked row: keep diagonal for a valid softmax
        mask_info.append(tuple(row))
    mask_info = tuple(mask_info)

    # pack mask-add tiles in (block, jt, il) order: a block's mixed tiles
    # for one key-tile are consecutive slots -> single wide DVE add
    am_slots = {}
    strips = []
    slot = 0
    for ig in range(4):
        for jt in range(NJT):
            for il in range(4):
                it = 4 * ig + il
                if mask_info[it][jt] == 1:
                    am_slots[(it, jt)] = slot
                    blk = am_p[it * 128:(it + 1) * 128,
                               jt * 128:(jt + 1) * 128]
                    # TRANSPOSED int8 indicator (1 = masked), [j, i] layout
                    strips.append(np.ascontiguousarray(
                        (blk.T < -1e8)).astype(np.int8))
                    slot += 1
    n_am = slot
    if n_am:
        am_np = np.ascontiguousarray(
            np.stack(strips, axis=1))  # [128, n_am, 128]
    else:
        am_np = np.zeros((128, 1, 128), dtype=np.int8)
    am_slots = tuple(sorted(am_slots.items()))

    # hidden states: [p, kt, t] bf16
    hs_np = np.ascontiguousarray(
        hs_p.T.reshape(NKT, 128, S).transpose(1, 0, 2)
    ).astype(ml_dtypes.bfloat16)

    in_maps = []
    for cid in range(NCORES):
        # wqkv tile layout [nb, p, e, kt, n]; nb = 3*hl + part
        wq_np = np.empty((NBLK, 128, 2, NKT, 128), dtype=ml_dtypes.bfloat16)
        for hl in range(HPC):
            h = HPC * cid + hl
            for part in range(3):
                nb = 3 * hl + part
                col0 = part * H + h * HD
                for e, w in ((0, wv_qkv), (1, wl_qkv)):
                    blk = w[:, col0:col0 + HD]  # [4096, 128]
                    wq_np[nb, :, e] = blk.reshape(NKT, 128, 128).transpose(
                        1, 0, 2).astype(ml_dtypes.bfloat16)
        # wdense tile layout [nb, p, e, dt, n]
        r0 = 512 * cid
        wd_np = np.empty((32, 128, 2, HPC, 128), dtype=ml_dtypes.bfloat16)
        for e, w in ((0, wv_dense), (1, wl_dense)):
            blk = w[r0:r0 + 512]  # [512, 4096]
            wd_np[:, :, e] = blk.reshape(HPC, 128, 32, 128).transpose(
                2, 1, 0, 3).astype(ml_dtypes.bfloat16)
        in_maps.append({
            "hs": hs_np,
            "wqkv": np.ascontiguousarray(wq_np),
            "wdense": np.ascontiguousarray(wd_np),
            "cosT": cosT,
            "sinT": sinT,
            "amask": am_np,
        })
    key = (ranges, mask_info, am_slots, n_am)
    return key, perm, in_maps


PROFILE = False
LAST_EXEC_NS = None
LAST_RESULTS = None


def kernel(**inputs):
    global LAST_EXEC_NS, LAST_RESULTS
    from concourse.bass_utils import run_bass_kernel_spmd

    key, perm, in_maps = _host_prep(inputs)
    bkey = (key[0], key[1], key[3])
    if bkey not in _CACHE:
        am_slots = dict(key[2])
        _CACHE[bkey] = _build(key[0], key[1], am_slots, key[3])
    nc = _CACHE[bkey]
    kw = {}
    if PROFILE:
        try:
            import antenv.axon_hooks  # noqa: F401
            kw = {"trace": True}
        except ImportError:
            pass
    res = run_bass_kernel_spmd(nc, in_maps, core_ids=list(range(NCORES)), **kw)
    LAST_EXEC_NS = res.exec_time_ns
    LAST_RESULTS = res
    acc = np.zeros((32, 128, S), dtype=np.float32)
    for r in res.results:
        acc += np.asarray(r["outT"], dtype=np.float32)
    full = acc.reshape(H, S)  # [h, t]
    out = np.empty((S, H), dtype=np.float32)
    out[perm] = full.T
    return np.ascontiguousarray(out).reshape(B, S, H)


# revision 75
# speedup vs baseline: 1.0040x; 1.0038x over previous
"""CogVLM vision-expert attention on 8 Trainium2 NeuronCores.

Sharding: tensor-parallel over heads (4 heads per core). Each core gets
- replicated: hidden_states (transposed, bf16), RoPE tables, mask tiles
- sharded:    QKV weight columns + dense weight rows for its 4 heads
Each core computes q/k/v for its heads, head-local attention, and a
row-parallel partial of the dense output; the host sums the 8 partials.

Expert routing: tokens are host-sorted language-first, so the two experts
become an exact column split at the sorted boundary — every matmul computes
each token's single expert, no select/merge needed on device.

All matmul operands are bf16 (f32 PSUM accumulation). Attention runs in
transposed (S^T) orientation per block of 4 query tiles: scores land
[key, query] in PSUM, exp writes P^T straight to SBUF (no transposes, no
extra copies), row sums come from 1-row matmuls P_tile.T @ ones, and the
1/rowsum lands as a per-partition scale when ctx^T[i, d] is copied out.

Self-contained: hardcodes all shapes; only needs numpy + concourse.
"""

import numpy as np

B, S, H, NH = 1, 2048, 4096, 32
HD = H // NH          # 128
NCORES = 8
HPC = NH // NCORES    # 4 heads per core
NBLK = 3 * HPC        # 12 qkv col-blocks of 128 per core
NKT = H // 128        # 32 contraction tiles
NJT = S // 128        # 16 token tiles
ROPE_BASE = 10000.0

_CACHE = {}

# stage-B tuning knobs (engine: 'v'=DVE, 'g'=GPSIMD/Pool)
TUNE = {
    "sp_bufs": 4,      # PSUM score tiles in flight
    "cps_bufs": 2,
    "ctp_bufs": 1,
}


def _f32(x):
    return np.ascontiguousarray(x, dtype=np.float32)


def _build(ranges, mask_info, am_slots, n_am, dbg=False):
    """ranges: per 512-chunk tuple of (a, b, e) column ranges (exact expert
    split). mask_info[it][jt]: 0=compute, 1=compute+mask-add, 2=skip.
    am_slots: {(it, jt): slot} for mask-add tiles packed on host."""
    import concourse.bass as bass
    import concourse.mybir as mybir
    import concourse.tile as tile
    from concourse import bacc
    from contextlib import ExitStack
    import ml_dtypes

    dt = mybir.dt
    f32, bf16, f16 = dt.float32, dt.bfloat16, dt.float16
    AF = mybir.ActivationFunctionType
    AX = mybir.AxisListType.X

    nc = bacc.Bacc("TRN2", target_bir_lowering=False, debug=False)

    # DRAM inputs (host-packed layouts; see _host_prep)
    hs_d = nc.dram_tensor("hs", [128, NKT, S], bf16, kind="ExternalInput")
    wq_d = nc.dram_tensor("wqkv", [NBLK, 128, 2, NKT, 128], bf16,
                          kind="ExternalInput")
    wd_d = nc.dram_tensor("wdense", [32, 128, 2, HPC, 128], bf16,
                          kind="ExternalInput")
    cos_d = nc.dram_tensor("cosT", [HD, S], bf16, kind="ExternalInput")
    sin_d = nc.dram_tensor("sinT", [HD, S], bf16, kind="ExternalInput")
    am_d = nc.dram_tensor("amask", [128, max(n_am, 1), 128], dt.int8,
                          kind="ExternalInput")
    out_d = nc.dram_tensor("outT", [32, 128, S], bf16, kind="ExternalOutput")
    if dbg:
        qkv_dump = nc.dram_tensor("qkv_dump", [128, NBLK, S], bf16,
                                  kind="ExternalOutput")
        ctx_dump = nc.dram_tensor("ctx_dump", [HPC, 128, S], bf16,
                                  kind="ExternalOutput")

    eye_bf16 = nc.inline_tensor(np.eye(128, dtype=ml_dtypes.bfloat16),
                                "eye_bf16")
    # RT.T @ qT = rotate_half(q): row d<64 = -q[d+64], d>=64 = +q[d-64]
    RT_np = np.zeros((128, 128), dtype=ml_dtypes.bfloat16)
    for j in range(64):
        RT_np[j, j + 64] = 1.0
        RT_np[j + 64, j] = -1.0
    RT_t = nc.inline_tensor(RT_np, "RT")

    # per-it: maximal runs of non-skip tiles (for exp), and runs chopped to
    # <=8 jt (for PSUM score tiles)
    runs_of = []
    segs_of = []
    for it in range(NJT):
        runs = []
        segs = []
        j = 0
        while j < NJT:
            if mask_info[it][j] == 2:
                j += 1
                continue
            j0 = j
            while j < NJT and mask_info[it][j] != 2:
                j += 1
            runs.append((j0, j))
            for s0 in range(j0, j, 8):
                segs.append((s0, min(s0 + 8, j)))
        runs_of.append(runs)
        segs_of.append(segs)
    tiles_of = [[jt for jt in range(NJT) if mask_info[it][jt] != 2]
                for it in range(NJT)]
    # mask-add runs per it: consecutive mixed jts share consecutive slots
    am_runs = []
    for it in range(NJT):
        runs = []
        j = 0
        while j < NJT:
            if mask_info[it][j] != 1:
                j += 1
                continue
            j0 = j
            while j < NJT and mask_info[it][j] == 1:
                j += 1
            runs.append((j0, j, am_slots[(it, j0)]))
        am_runs.append(runs)

    with tile.TileContext(nc) as tc, ExitStack() as top:
        singles = top.enter_context(tc.tile_pool(name="singles", bufs=1))
        ident16 = singles.tile([128, 128], bf16)
        nc.sync.dma_start(out=ident16, in_=eye_bf16[:, :])
        RT_sb = singles.tile([128, 128], bf16)
        nc.sync.dma_start(out=RT_sb, in_=RT_t[:, :])
        nbias = singles.tile([128, 1], f32)
        nc.vector.memset(nbias, -24.0)

        # qkv (bf16) persists across stages A and B
        qkv_sb = singles.tile([128, NBLK, S], bf16, name="qkv_sb")

        # ---------------- Stage A: QKV projection (exact expert split) -----
        with ExitStack() as sa:
            pa = sa.enter_context(tc.tile_pool(name="qkv_sbuf", bufs=1))
            ppa = sa.enter_context(tc.tile_pool(name="qkv_psum", bufs=1,
                                                space="PSUM"))
            hs_sb = pa.tile([128, NKT, S], bf16, name="hs_sb")
            # 2D-sliced sub-DMAs (kt-groups x 512-token columns) in the
            # order the first chunk's kt-loop consumes them
            for tc_ in range(4):
                for kg in range(4):
                    nc.sync.dma_start(
                        out=hs_sb[:, kg * 8:(kg + 1) * 8,
                                  tc_ * 512:(tc_ + 1) * 512],
                        in_=hs_d[:, kg * 8:(kg + 1) * 8,
                                 tc_ * 512:(tc_ + 1) * 512])

            KH = NKT // 2  # weight tiles split in kt-halves to fit SBUF

            def load_w(nb):
                halves = []
                for kh in range(2):
                    w = pa.tile([128, 2, KH, 128], bf16, tag="w", bufs=3,
                                name=f"w_{nb}_{kh}")
                    nc.gpsimd.dma_start(
                        out=w, in_=wq_d[nb, :, :, kh * KH:(kh + 1) * KH, :])
                    halves.append(w)
                return halves

            next_w = load_w(0)
            for nb in range(NBLK):
                w = next_w
                if nb + 1 < NBLK:
                    next_w = load_w(nb + 1)
                for c in range(4):
                    ps = ppa.tile([128, 512], f32, tag="mmA", bufs=6,
                                  name=f"ps_{nb}_{c}")
                    for (a, b, e) in ranges[c]:
                        for kt in range(NKT):
                            nc.tensor.matmul(
                                ps[:, a:b],
                                lhsT=w[kt // KH][:, e, kt % KH, :],
                                rhs=hs_sb[:, kt, c * 512 + a:c * 512 + b],
                                start=(kt == 0), stop=(kt == NKT - 1),
                            )
                    nc.vector.tensor_copy(
                        out=qkv_sb[:, nb, c * 512:(c + 1) * 512], in_=ps)

        if dbg:
            nc.sync.dma_start(out=qkv_dump[:, :, :], in_=qkv_sb)

        # ctx tiles live across stages B and C (allocated after A frees hs)
        bc_pool = top.enter_context(tc.tile_pool(name="bc", bufs=1))
        ctxT = [bc_pool.tile([128, S], bf16, name=f"ctxT_{hl}")
                for hl in range(HPC)]

        # ---------------- Stage B: per-head attention ----------------------
        with ExitStack() as sb:
            pb = sb.enter_context(tc.tile_pool(name="att_sbuf", bufs=1))
            ppb = sb.enter_context(tc.tile_pool(name="att_psum", bufs=1,
                                                space="PSUM"))
            cos_sb = pb.tile([HD, S], bf16, name="cos_sb")
            nc.gpsimd.dma_start(out=cos_sb, in_=cos_d[:, :])
            sin_sb = pb.tile([HD, S], bf16, name="sin_sb")
            nc.gpsimd.dma_start(out=sin_sb, in_=sin_d[:, :])
            am_sb = None
            if n_am:
                am_sb = pb.tile([128, n_am, 128], dt.int8, name="am_sb")
                nc.gpsimd.dma_start(out=am_sb, in_=am_d[:, :, :])

            ENG = {"v": nc.vector, "g": nc.gpsimd}
            ei = [0]

            def rr(spec):  # round-robin over an engine spec string
                e = ENG[spec[ei[0] % len(spec)]]
                ei[0] += 1
                return e

            # RoPE: x' = x*cos + (RT.T @ x)*sin (scales folded into cos/sin
            # on host). All-bf16 DVE ops for 2x/4x DVE throughput. Head 0's
            # chunks are emitted upfront; heads 1-3's chunks interleave into
            # the previous head's it-loop so the DVE chain hides under PE.
            qkr = [(pb.tile([128, S], bf16, name=f"qr{hl}"),
                    pb.tile([128, S], bf16, name=f"kr{hl}"))
                   for hl in range(HPC)]

            def emit_rope_chunk(hl, xb, xr, ch, tag="cps"):
                cs = slice(ch * 512, ch * 512 + 512)
                if tag == "cps":
                    rot_t = ppb.tile([128, 4, 128], f32, tag="cps",
                                     bufs=TUNE["cps_bufs"],
                                     name=f"rot{hl}_{xb}_{ch}")
                    rot = rot_t[:, :, :]
                else:
                    rot = ppb.tile([128, 512], f32, tag="mmB",
                                   bufs=TUNE["sp_bufs"],
                                   name=f"rot{hl}_{xb}_{ch}")
                nc.tensor.matmul(rot, lhsT=RT_sb, rhs=qkv_sb[:, xb, cs],
                                 start=True, stop=True)
                m1 = pb.tile([128, 512], bf16, tag="ropetmp", bufs=3,
                             name=f"m1_{hl}_{xb}_{ch}")
                nc.vector.tensor_mul(out=m1, in0=qkv_sb[:, xb, cs],
                                     in1=cos_sb[:, cs])
                m2 = pb.tile([128, 512], bf16, tag="ropetmp2", bufs=3,
                             name=f"m2_{hl}_{xb}_{ch}")
                nc.vector.tensor_mul(out=m2, in0=rot, in1=sin_sb[:, cs])
                nc.vector.tensor_add(out=xr[:, cs], in0=m1, in1=m2)

            from collections import deque
            rope_q = deque()
            for hl in range(HPC):
                for xb, xr in ((3 * hl, qkr[hl][0]), (3 * hl + 1, qkr[hl][1])):
                    for ch in range(4):
                        rope_q.append((hl, xb, xr, ch))
            for _ in range(8):  # head 0 upfront: mmB tag is idle here
                emit_rope_chunk(*rope_q.popleft(), tag="mmB")

            ones_sb = pb.tile([128, 1], bf16, name="ones_col")
            nc.vector.memset(ones_sb, 1.0)
            zeros_sb = pb.tile([128, 4, 128], bf16, name="zeros_sb")
            nc.vector.memset(zeros_sb, 0.0)

            pending = [None]  # previous i-block awaiting PV/finish
            for hl in range(HPC):
                bq, bk, bv = 3 * hl, 3 * hl + 1, 3 * hl + 2
                qr, kr = qkr[hl]

                # v -> [t, d] layout via PE transpose; batches emitted
                # inside ig0's QK loop so they do not burst at head start
                v_sb = pb.tile([128, NJT, 128], bf16, tag="v_sb", bufs=2,
                               name=f"v{hl}")

                def emit_vt(jg, v_sb=v_sb, bv=bv):
                    vtp = ppb.tile([128, 4, 128], bf16, tag="ctp",
                                   bufs=TUNE["ctp_bufs"],
                                   name=f"vt{hl}_{jg}")
                    for j in range(4):
                        nc.tensor.transpose(
                            vtp[:, j, :],
                            qkv_sb[:, bv, (jg * 4 + j) * 128:
                                   (jg * 4 + j + 1) * 128],
                            ident16)
                    nc.vector.tensor_copy(
                        out=v_sb[:, jg * 4:(jg + 1) * 4, :], in_=vtp)

                # Attention in S^T orientation per block of 4 i-tiles:
                # scores land [j, i]; exp writes P^T straight to SBUF; row
                # sums via 1-row matmuls P_tile.T @ ones; 1/sum applied
                # per-partition when ctx^T[i, d] copies out; ctx then
                # re-transposed to [d, i]. The previous block's PV work is
                # emitted in il-sequential chunks inside the current block's
                # QK/exp phase (sequential PSUM accumulation groups).

                def emit_pv_op(st, il, jt):
                    til = tiles_of[st["block"][il]]
                    sl = st["slot"][jt]
                    nc.tensor.matmul(
                        st["rsum"][:, il:il + 1],
                        lhsT=st["pT_sb"][:, sl, il * 128:(il + 1) * 128],
                        rhs=ones_sb,
                        start=(jt == til[0]), stop=(jt == til[-1]))
                    nc.tensor.matmul(
                        st["cps"][:, il, :],
                        lhsT=st["pT_sb"][:, sl, il * 128:(il + 1) * 128],
                        rhs=st["v_sb"][:, jt, :],
                        start=(jt == til[0]), stop=(jt == til[-1]))

                def pv_ops_of(st):
                    return [(il, jt) for il in range(4)
                            for jt in tiles_of[st["block"][il]]]

                def emit_finish(st):
                    hl_, ig_ = st["hl"], st["ig"]
                    rec_col = pb.tile([128, 4], f32, tag="rec", bufs=2,
                                      name=f"rec{hl_}_{ig_}")
                    nc.vector.reciprocal(out=rec_col, in_=st["rsum"])
                    ctxi = pb.tile([128, 4, 128], bf16, tag="ctxi", bufs=2,
                                   name=f"ctxi{hl_}_{ig_}")
                    for il in range(4):
                        nc.vector.tensor_scalar_mul(
                            out=ctxi[:, il, :], in0=st["cps"][:, il, :],
                            scalar1=rec_col[:, il:il + 1])
                    ctp = ppb.tile([128, 4, 128], bf16, tag="ctp",
                                   bufs=TUNE["ctp_bufs"],
                                   name=f"ctp{hl_}_{ig_}")
                    for il in range(4):
                        nc.tensor.transpose(ctp[:, il, :], ctxi[:, il, :],
                                            ident16)
                    nc.vector.tensor_copy(
                        out=ctxT[hl_][:, st["i0"]:st["i0"] + 512], in_=ctp)

                for ig in range(4):
                    block = [4 * ig + il for il in range(4)]
                    jts = sorted(set().union(
                        *[set(tiles_of[it]) for it in block]))
                    i0 = ig * 512
                    rsum = ppb.tile([128, 4], f32, tag="rsum", bufs=1,
                                    name=f"rsum{hl}_{ig}")
                    pT_sb = pb.tile([128, NJT, 512], bf16, tag="pT", bufs=3,
                                    name=f"pT{hl}_{ig}")
                    cps = ppb.tile([128, 4, 128], f32, tag="cps",
                                   bufs=TUNE["cps_bufs"],
                                   name=f"cps{hl}_{ig}")

                    def emit_qkT(k, jts=jts, block=block, i0=i0,
                                 pT_sb=pT_sb):
                        jt = jts[k]
                        # only the valid query-tile range needs computing;
                        # invalid slots inside it are never read downstream
                        valid = [il for il, it in enumerate(block)
                                 if mask_info[it][jt] != 2]
                        lo, hi = valid[0], valid[-1] + 1
                        spT = ppb.tile([128, 512], f32, tag="mmB",
                                       bufs=TUNE["sp_bufs"],
                                       name=f"spT{hl}_{ig}_{jt}")
                        nc.tensor.matmul(
                            spT[:, lo * 128:hi * 128],
                            lhsT=kr[:, jt * 128:(jt + 1) * 128],
                            rhs=qr[:, i0 + lo * 128:i0 + hi * 128],
                            start=True, stop=True)
                        nc.scalar.activation(
                            out=pT_sb[:, k, lo * 128:hi * 128],
                            in_=spT[:, lo * 128:hi * 128], func=AF.Exp,
                            bias=nbias, scale=1.0)
                        # zero masked entries after exp (mask tiles are
                        # nonzero exactly where masked); off the spT chain
                        il = 0
                        while il < 4:
                            if mask_info[block[il]][jt] != 1:
                                il += 1
                                continue
                            a = il
                            while il < 4 and mask_info[block[il]][jt] == 1:
                                il += 1
                            slot = am_slots[(block[a], jt)]
                            nc.vector.copy_predicated(
                                out=pT_sb[:, k, a * 128:il * 128],
                                mask=am_sb[:, slot:slot + (il - a), :],
                                data=zeros_sb[:, :il - a, :])

                    pvq = pv_ops_of(pending[0]) if pending[0] else []
                    per_k = max(1, -(-len(pvq) // max(1, len(jts))))
                    pi = 0
                    for k in range(len(jts)):
                        emit_qkT(k)
                        if ig == 0 and k < 4:
                            emit_vt(k)
                        if k % 4 == 1 and rope_q and rope_q[0][0] == hl + 1:
                            emit_rope_chunk(*rope_q.popleft())
                        for _ in range(per_k):
                            if pi < len(pvq):
                                emit_pv_op(pending[0], *pvq[pi])
                                pi += 1
                    if ig == 0:
                        for jg in range(len(jts), 4):
                            emit_vt(jg)
                    if pending[0] is not None:
                        while pi < len(pvq):
                            emit_pv_op(pending[0], *pvq[pi])
                            pi += 1
                        emit_finish(pending[0])
                    pending[0] = dict(hl=hl, ig=ig, block=block, jts=jts,
                                      i0=i0, rsum=rsum, pT_sb=pT_sb,
                                      cps=cps, v_sb=v_sb,
                                      slot={jt: kk for kk, jt
                                            in enumerate(jts)})

            # flush the last block
            st = pending[0]
            for il, jt in pv_ops_of(st):
                emit_pv_op(st, il, jt)
            emit_finish(st)
            pending[0] = None

        if dbg:
            for hl in range(HPC):
                nc.sync.dma_start(out=ctx_dump[hl], in_=ctxT[hl])

        # ---------------- Stage C: row-parallel dense (exact split) --------
        with ExitStack() as sc:
            # C's SBUF tiles live in bc_pool (allocated before stage B claims
            # space): no WAR against B's tail readers, prefetch starts early
            pc = bc_pool
            ppc = sc.enter_context(tc.tile_pool(name="dense_psum", bufs=1,
                                                space="PSUM"))

            def load_wd(nb):
                wd = pc.tile([128, 2, HPC, 128], bf16, tag="wd", bufs=6,
                             name=f"wd_{nb}")
                nc.gpsimd.dma_start(out=wd, in_=wd_d[nb])
                return wd

            next_wd = load_wd(0)
            for nb in range(32):
                wd = next_wd
                if nb + 1 < 32:
                    next_wd = load_wd(nb + 1)
                ob = pc.tile([128, S], bf16, tag="ob", bufs=3,
                             name=f"ob_{nb}")
                for c in range(4):
                    ops = ppc.tile([128, 512], f32, tag="mmC", bufs=6,
                                   name=f"o_{nb}_{c}")
                    for (a, b, e) in ranges[c]:
                        for dt_ in range(HPC):
                            nc.tensor.matmul(
                                ops[:, a:b],
                                lhsT=wd[:, e, dt_, :],
                                rhs=ctxT[dt_][:, c * 512 + a:c * 512 + b],
                                start=(dt_ == 0), stop=(dt_ == HPC - 1))
                    nc.scalar.activation(
                        out=ob[:, c * 512:(c + 1) * 512], in_=ops,
                        func=AF.Copy, bias=0.0, scale=1.0)
                nc.gpsimd.dma_start(out=out_d[nb], in_=ob)

    nc.finalize()
    return nc


def _host_prep(inputs):
    import ml_dtypes

    hs = _f32(np.asarray(inputs["hidden_states"])).reshape(S, H)
    tt = np.asarray(inputs["token_type_ids"]).reshape(S)
    pos = np.asarray(inputs["position_ids"]).reshape(S).astype(np.int64)
    am = _f32(np.asarray(inputs["attention_mask"])).reshape(
        np.asarray(inputs["attention_mask"]).shape[-2], -1
    )[:S, :S]
    wv_qkv = _f32(inputs["wv_qkv"])
    wl_qkv = _f32(inputs["wl_qkv"])
    wv_dense = _f32(inputs["wv_dense"])
    wl_dense = _f32(inputs["wl_dense"])

    # routing mask: vision iff tt[i]==1 and tt[i+1]==1; last position language
    core = (tt[:-1] == 1) & (tt[1:] == 1)
    vmb = np.concatenate([core, [False]])

    # sort tokens: language first, stable; attention uses the permuted mask
    perm = np.argsort(vmb, kind="stable")
    vmb_p = vmb[perm]
    nl = int((~vmb_p).sum())  # tokens [0, nl) language (expert 1), rest vision
    hs_p = hs[perm]
    pos_p = pos[perm]
    am_p = np.ascontiguousarray(am[np.ix_(perm, perm)])

    # exact expert column ranges per 512-token chunk (e: 0=vision, 1=language)
    ranges = []
    for c in range(4):
        lo, hi = 512 * c, 512 * (c + 1)
        if hi <= nl:
            ranges.append(((0, 512, 1),))
        elif lo >= nl:
            ranges.append(((0, 512, 0),))
        else:
            ranges.append(((0, nl - lo, 1), (nl - lo, 512, 0)))
    ranges = tuple(ranges)

    inv_freq = 1.0 / (ROPE_BASE ** (np.arange(0, HD, 2, dtype=np.float32) / HD))
    t = np.arange(S, dtype=np.float32)
    emb = np.concatenate([np.outer(t, inv_freq)] * 2, axis=-1)  # [S, HD]
    ss = np.float32(np.sqrt(1.0 / np.sqrt(HD)))
    cosT = np.ascontiguousarray(
        (np.cos(emb).astype(np.float32) * ss)[pos_p].T).astype(
        ml_dtypes.bfloat16)  # [HD, S]
    sinT = np.ascontiguousarray(
        (np.sin(emb).astype(np.float32) * ss)[pos_p].T).astype(
        ml_dtypes.bfloat16)

    # per-(i-tile, j-tile) mask: 0=all-zero, 1=mixed (add), 2=all-masked (skip)
    mask_info = []
    for it in range(NJT):
        row = []
        for jt in range(NJT):
            blk = am_p[it * 128:(it + 1) * 128, jt * 128:(jt + 1) * 128]
            if blk.max() < -1e8:
                row.append(2)
            elif blk.min() == 0.0 and blk.max() == 0.0:
                row.append(0)
            else:
                row.append(1)
        if all(s == 2 for s in row):
            row[it] = 1  # fully-masked row: keep diagonal for a valid softmax
        mask_info.append(tuple(row))
    mask_info = tuple(mask_info)

    # pack mask-add tiles in (block, jt, il) order: a block's mixed tiles
    # for one key-tile are consecutive slots -> single wide DVE add
    am_slots = {}
    strips = []
    slot = 0
    for ig in range(4):
        for jt in range(NJT):
            for il in range(4):
                it = 4 * ig + il
                if mask_info[it][jt] == 1:
                    am_slots[(it, jt)] = slot
                    blk = am_p[it * 128:(it + 1) * 128,
                               jt * 128:(jt + 1) * 128]
                    # TRANSPOSED int8 indicator (1 = masked), [j, i] layout
                    strips.append(np.ascontiguousarray(
                        (blk.T < -1e8)).astype(np.int8))
                    slot += 1
    n_am = slot
    if n_am:
        am_np = np.ascontiguousarray(
            np.stack(strips, axis=1))  # [128, n_am, 128]
    else:
        am_np = np.zeros((128, 1, 128), dtype=np.int8)
    am_slots = tuple(sorted(am_slots.items()))

    # hidden states: [p, kt, t] bf16
    hs_np = np.ascontiguousarray(
        hs_p.T.reshape(NKT, 128, S).transpose(1, 0, 2)
    ).astype(ml_dtypes.bfloat16)

    in_maps = []
    for cid in range(NCORES):
        # wqkv tile layout [nb, p, e, kt, n]; nb = 3*hl + part
        wq_np = np.empty((NBLK, 128, 2, NKT, 128), dtype=ml_dtypes.bfloat16)
        for hl in range(HPC):
            h = HPC * cid + hl
            for part in range(3):
                nb = 3 * hl + part
                col0 = part * H + h * HD
                for e, w in ((0, wv_qkv), (1, wl_qkv)):
                    blk = w[:, col0:col0 + HD]  # [4096, 128]
                    wq_np[nb, :, e] = blk.reshape(NKT, 128, 128).transpose(
                        1, 0, 2).astype(ml_dtypes.bfloat16)
        # wdense tile layout [nb, p, e, dt, n]
        r0 = 512 * cid
        wd_np = np.empty((32, 128, 2, HPC, 128), dtype=ml_dtypes.bfloat16)
        for e, w in ((0, wv_dense), (1, wl_dense)):
            blk = w[r0:r0 + 512]  # [512, 4096]
            wd_np[:, :, e] = blk.reshape(HPC, 128, 32, 128).transpose(
                2, 1, 0, 3).astype(ml_dtypes.bfloat16)
        in_maps.append({
            "hs": hs_np,
            "wqkv": np.ascontiguousarray(wq_np),
            "wdense": np.ascontiguousarray(wd_np),
            "cosT": cosT,
            "sinT": sinT,
            "amask": am_np,
        })
    key = (ranges, mask_info, am_slots, n_am)
    return key, perm, in_maps


PROFILE = False
LAST_EXEC_NS = None
LAST_RESULTS = None


def kernel(**inputs):
    global LAST_EXEC_NS, LAST_RESULTS
    from concourse.bass_utils import run_bass_kernel_spmd

    key, perm, in_maps = _host_prep(inputs)
    bkey = (key[0], key[1], key[3])
    if bkey not in _CACHE:
        am_slots = dict(key[2])
        _CACHE[bkey] = _build(key[0], key[1], am_slots, key[3])
    nc = _CACHE[bkey]
    kw = {}
    if PROFILE:
        try:
            import antenv.axon_hooks  # noqa: F401
            kw = {"trace": True}
        except ImportError:
            pass
    res = run_bass_kernel_spmd(nc, in_maps, core_ids=list(range(NCORES)), **kw)
    LAST_EXEC_NS = res.exec_time_ns
    LAST_RESULTS = res
    acc = np.zeros((32, 128, S), dtype=np.float32)
    for r in res.results:
        acc += np.asarray(r["outT"], dtype=np.float32)
    full = acc.reshape(H, S)  # [h, t]
    out = np.empty((S, H), dtype=np.float32)
    out[perm] = full.T
    return np.ascontiguousarray(out).reshape(B, S, H)
